# revision 8
# baseline (speedup 1.0000x reference)
"""Contrastive loss (NT-Xent) on 8 Trainium2 cores — v2.

Per-core layout: inputs are the full feature-major x^T cast to fp8e4 on host,
ROTATED by -c*1024 columns so each core's own 1024 rows sit at local columns
[0, 1024) and the positive-pair block at [4096, 5120) (static offsets, one
SPMD program).  Row sums are rotation-invariant.

Device pipeline per core:
  squares (DVE, bf16) -> column sumsq via ones-matmul (PE, f32 PSUM)
  -> magic-rsqrt*16 with 1 Newton step (DVE, int ops) -> r broadcast via DMA
  -> z8 = fp8(x8 * rbc)  (DVE)
  -> sim row-block GEMM in fp8 DoubleRow (256-contraction per pass, PE)
  -> exp(sim*10/256) row-sums split across engines:
       ACT: true exp via activation(accum_out)
       DVE/Pool: Schraudolph fast-exp (affine->int32->bitcast) + reduce
  -> diag correction (+1 - exp(selfsim*k)), pos term, log, partial loss scalar.
Host sums 8 partials / 2N.
"""

import numpy as np
import ml_dtypes

import concourse.bass as bass
import concourse.tile as tile
from concourse import bacc, mybir

F32 = mybir.dt.float32
F32R = mybir.dt.float32r
BF16 = mybir.dt.bfloat16
FP8 = mybir.dt.float8e4
I32 = mybir.dt.int32
I16 = mybir.dt.int16
AF = mybir.ActivationFunctionType
ALU = mybir.AluOpType
AX = mybir.AxisListType
PSUM = bass.MemorySpace.PSUM
DR = mybir.MatmulPerfMode.DoubleRow

N = 4096
TWO_N = 2 * N
D = 256
RPC = TWO_N // 8          # rows per core = 1024
M_TILES = RPC // 128      # 8 row tiles per core
G = 4                     # column groups
GW = TWO_N // G           # 2048

TAU_INV = 10.0
SCALE_Z = 16.0
K_SIM = TAU_INV / (SCALE_Z * SCALE_Z)          # exp scale on raw GEMM output
LOG2E = 1.4426950408889634
S32 = float(np.float32((2.0**23) * LOG2E * K_SIM))
B32 = float(np.float32((2.0**23) * 127 - 470000.0))   # tuned in model.py
S16 = float(np.float32((2.0**7) * LOG2E * K_SIM))
B16 = float(np.float32((2.0**7) * 127 - 470000.0 / (2.0**16)))
MAGIC16 = 0x5F3759DF + 0x02000000                     # rsqrt magic, *16 folded

# main-loop exp engine per tile (t = g*8+m): A=scalar-act, D=vector
# (gpsimd cannot read PSUM on trn2, so Pool gets prep work instead)
TILE_ENGINE = ["A"] * (G * M_TILES)
for _t in (9, 12, 15, 18, 21, 24, 27, 30):
    TILE_ENGINE[_t] = "D"

RHS_W = 512               # DR out width per matmul (one PSUM bank)


def build_nc(nc=None):
    if nc is None:
        nc = bacc.Bacc("TRN2", target_bir_lowering=False, debug=False)

    xt_d = [
        nc.declare_dram_parameter(f"xt{k}", [128, TWO_N], FP8, isOutput=False)
        for k in range(2)
    ]
    out_d = nc.declare_dram_parameter("out", [1, 1], F32, isOutput=True)

    with tile.TileContext(nc) as tc:
        with (
            tc.tile_pool(name="const", bufs=1) as cpool,
            tc.tile_pool(name="xt", bufs=1) as xt_pool,
            tc.tile_pool(name="zt", bufs=1) as zt_pool,
            tc.tile_pool(name="xsq", bufs=4) as xsq_pool,
            tc.tile_pool(name="rsq", bufs=2) as rsq_pool,
            tc.tile_pool(name="rbc", bufs=4) as rbc_pool,
            tc.tile_pool(name="ti", bufs=2) as ti_pool,
            tc.tile_pool(name="tip", bufs=2) as tip_pool,
            tc.tile_pool(name="junk", bufs=1) as junk_pool,
            tc.tile_pool(name="fin", bufs=1) as fin_pool,
            tc.tile_pool(name="dram", bufs=1, space="DRAM") as dram_pool,
        ):
            ones_bf = cpool.tile([128, 1], BF16, name="ones_bf", tag="ones_bf")
            nc.vector.memset(ones_bf[:], 1.0)
            ones_f32 = cpool.tile([128, 1], F32, name="ones_f32", tag="ones_f32")
            nc.vector.memset(ones_f32[:], 1.0)

            xt_t = [
                xt_pool.tile([128, TWO_N], FP8, name=f"xts{k}", tag=f"xts{k}")
                for k in range(2)
            ]
            zt8 = zt_pool.tile([128, 2, TWO_N], FP8, name="zt8", tag="zt8")

            den_acc = fin_pool.tile([128, M_TILES * G], F32, name="den_acc",
                                    tag="den_acc")
            selfexp_t = fin_pool.tile([128, M_TILES], F32, name="selfexp_t",
                                      tag="selfexp_t")
            possum = fin_pool.tile([1, 1], F32, name="possum", tag="possum")

            ssq_dram = dram_pool.tile([1, TWO_N], F32, name="ssq_dram",
                                      tag="ssq_dram")
            r_dram = dram_pool.tile([1, TWO_N], BF16, name="r_dram", tag="r_dram")
            se_dram = dram_pool.tile([1, RPC], F32, name="se_dram", tag="se_dram")

            # ---- input DMAs (all groups queued up front) ----
            for g in range(G):
                gs = slice(g * GW, (g + 1) * GW)
                for k in range(2):
                    nc.sync.dma_start(xt_t[k][:, gs], xt_d[k][:, gs])

            junk = (junk_pool.tile([128, GW], FP8, name="junk", tag="junk")
                    if "P" in TILE_ENGINE else None)

            def do_tile(g, m, pool):
                ms = slice(m * 128, (m + 1) * 128)
                st = pool.tile([128, GW], F32, name="sim", tag="sim")
                for j in range(GW // RHS_W):
                    cs = slice(g * GW + j * RHS_W, g * GW + (j + 1) * RHS_W)
                    nc.tensor.matmul(
                        st[:, j * RHS_W:(j + 1) * RHS_W],
                        zt8[:, :, ms], zt8[:, :, cs],
                        start=True, stop=True, perf_mode=DR)
                t = g * M_TILES + m
                dcol = den_acc[:, m * G + g:m * G + g + 1]
                if TILE_ENGINE[t] == "A":
                    nc.scalar.activation(st[:], st[:], AF.Exp,
                                         scale=K_SIM, accum_out=dcol)
                else:
                    ti = ti_pool.tile([128, GW], I16, name="ti", tag="ti")
                    nc.vector.tensor_scalar(ti[:], st[:], S16, B16,
                                            ALU.mult, ALU.add)
                    nc.vector.tensor_reduce(dcol, ti[:].bitcast(BF16),
                                            axis=AX.X, op=ALU.add)

            # ---- prep: three passes over groups to keep DVE queue unstalled ----
            with tc.tile_pool(name="ssp", bufs=2, space=PSUM) as ss_pool:
                ssg_t = []
                for g in range(G):
                    g0 = g * GW
                    # pass A: squares + sumsq ones-matmul + evacuate + bounce
                    xsq = [
                        xsq_pool.tile([128, GW], BF16, name=f"xsq{k}",
                                      tag=f"xsq{k}")
                        for k in range(2)
                    ]
                    for k in range(2):
                        sq_eng = nc.gpsimd if (g % 2 == 1 and k == 1) else nc.vector
                        sq_eng.tensor_mul(
                            xsq[k][:], xt_t[k][:, g0:g0 + GW],
                            xt_t[k][:, g0:g0 + GW])
                    ss_sb = rsq_pool.tile([1, GW], F32, name="ss_sb",
                                          tag="ss_sb")
                    for j in range(GW // 512):
                        js = slice(j * 512, (j + 1) * 512)
                        ss = ss_pool.tile([1, 512], F32, name="ss", tag="ss")
                        for k in range(2):
                            nc.tensor.matmul(ss[:], ones_bf[:],
                                             xsq[k][:, js],
                                             start=(k == 0), stop=(k == 1))
                        if g % 2 == 0:
                            nc.scalar.copy(ss_sb[0:1, js], ss[:])
                        else:
                            nc.vector.tensor_copy(ss_sb[0:1, js], ss[:])
                    nc.sync.dma_start(ssq_dram[0:1, g0:g0 + GW], ss_sb[:])
                    ssg = rsq_pool.tile([128, 16], F32, name="ssg", tag="ssg",
                                        bufs=4)
                    nc.sync.dma_start(
                        ssg[:],
                        ssq_dram[0:1, g0:g0 + GW].rearrange(
                            "o (p m) -> (o p) m", p=128))
                    ssg_t.append(ssg)
                for g in range(G):
                    # pass B: magic-rsqrt chain (DVE int ops) + r bounce out
                    g0 = g * GW
                    ssg = ssg_t[g]
                    sh = rsq_pool.tile([128, 16], I32, name="sh", tag="sh")
                    nc.vector.tensor_scalar(sh[:], ssg[:].bitcast(I32), 1, None,
                                            ALU.arith_shift_right)
                    y0i = rsq_pool.tile([128, 16], I32, name="y0i", tag="y0i")
                    nc.vector.tensor_scalar(y0i[:], sh[:], -1, MAGIC16,
                                            ALU.mult, ALU.add)
                    y2 = rsq_pool.tile([128, 16], F32, name="y2", tag="y2")
                    nc.vector.tensor_mul(y2[:], y0i[:].bitcast(F32),
                                         y0i[:].bitcast(F32))
                    sy2 = rsq_pool.tile([128, 16], F32, name="sy2", tag="sy2")
                    nc.vector.tensor_mul(sy2[:], ssg[:], y2[:])
                    w = rsq_pool.tile([128, 16], F32, name="w", tag="w")
                    nc.vector.tensor_scalar(w[:], sy2[:], -0.5 / 256.0, 1.5,
                                            ALU.mult, ALU.add)
                    rb = rsq_pool.tile([128, 16], BF16, name="rb", tag="rb")
                    nc.vector.tensor_mul(rb[:], y0i[:].bitcast(F32), w[:])
                    nc.sync.dma_start(
                        r_dram[0:1, g0:g0 + GW].rearrange(
                            "o (p m) -> (o p) m", p=128),
                        rb[:])
                for g in range(G):
                    # pass C: rbc broadcast + z8 = x8 * rbc (fp8 out)
                    g0 = g * GW
                    for half in range(2):
                        c0 = g0 + half * 1024
                        rbc = rbc_pool.tile([128, 1024], BF16, name="rbc",
                                            tag="rbc")
                        nc.sync.dma_start(
                            rbc[:],
                            r_dram[0:1, c0:c0 + 1024].broadcast_to((128, 1024)))
                        for k in range(2):
                            ml_eng = nc.vector if (half + k) % 2 == 0 else nc.gpsimd
                            ml_eng.tensor_mul(
                                zt8[:, k, c0:c0 + 1024],
                                xt_t[k][:, c0:c0 + 1024], rbc[:])

                prod_a = [
                    fin_pool.tile([128, RPC], BF16, name=f"prod_a{k}",
                                  tag=f"prod_a{k}")
                    for k in range(2)
                ]
                prod_s = [
                    fin_pool.tile([128, RPC], BF16, name=f"prod_s{k}",
                                  tag=f"prod_s{k}")
                    for k in range(2)
                ]
                for k in range(2):
                    nc.gpsimd.tensor_mul(prod_a[k][:], zt8[:, k, 0:RPC],
                                         zt8[:, k, N:N + RPC])
                    nc.gpsimd.tensor_mul(prod_s[k][:], zt8[:, k, 0:RPC],
                                         zt8[:, k, 0:RPC])

            # ---- main loop (double-buffered) ----
            with tc.tile_pool(name="simp", bufs=2, space=PSUM) as sim_pool:
                for g in range(G):
                    for m in range(M_TILES):
                        do_tile(g, m, sim_pool)

            # ---- finalize ----
            with tc.tile_pool(name="finp", bufs=1, space=PSUM) as fpsum:
                pos_ps = fpsum.tile([1, RPC], F32, name="pos", tag="pos")
                selfs_ps = fpsum.tile([1, RPC], F32, name="selfs", tag="selfs")
                for j in range(RPC // 512):
                    js = slice(j * 512, (j + 1) * 512)
                    for k in range(2):
                        nc.tensor.matmul(pos_ps[0:1, js], ones_bf[:],
                                         prod_a[k][:, js],
                                         start=(k == 0), stop=(k == 1))
                    for k in range(2):
                        nc.tensor.matmul(selfs_ps[0:1, js], ones_bf[:],
                                         prod_s[k][:, js],
                                         start=(k == 0), stop=(k == 1))
                nc.vector.tensor_reduce(possum[:], pos_ps[:], axis=AX.X,
                                        op=ALU.add)
                selfexp_row = fin_pool.tile([1, RPC], F32, name="selfexp_row",
                                            tag="selfexp_row")
                nc.scalar.activation(selfexp_row[:], selfs_ps[:], AF.Exp,
                                     scale=K_SIM)
                se_ps = fpsum.tile([128, M_TILES], F32, name="se_ps",
                                   tag="se_ps")
                for m in range(M_TILES):
                    nc.tensor.transpose(
                        se_ps[:, m:m + 1],
                        selfexp_row[0:1, m * 128:(m + 1) * 128],
                        ones_f32[0:1, 0:1])
                nc.vector.tensor_copy(selfexp_t[:], se_ps[:])

                den8 = fin_pool.tile([128, M_TILES], F32, name="den8",
                                     tag="den8")
                nc.vector.tensor_reduce(
                    den8[:],
                    den_acc[:].rearrange("p (m g) -> p m g", g=G),
                    axis=AX.X, op=ALU.add)
                denc = fin_pool.tile([128, M_TILES], F32, name="denc",
                                     tag="denc")
                nc.vector.scalar_tensor_tensor(
                    denc[:], in0=den8[:], scalar=1.0, in1=selfexp_t[:],
                    op0=ALU.add, op1=ALU.subtract)
                logden = fin_pool.tile([128, M_TILES], F32, name="logden",
                                       tag="logden")
                nc.scalar.activation(logden[:], denc[:], AF.Ln)
                red = fin_pool.tile([128, 1], F32, name="red", tag="red")
                nc.vector.tensor_reduce(red[:], logden[:], axis=AX.X,
                                        op=ALU.add)
                tot_ps = fpsum.tile([1, 1], F32, name="tot", tag="tot")
                nc.tensor.matmul(tot_ps[:], ones_f32[:], red[:], start=True,
                                 stop=True)
                res = fin_pool.tile([1, 1], F32, name="res", tag="res")
                nc.vector.scalar_tensor_tensor(
                    res[:], in0=possum[:], scalar=-K_SIM, in1=tot_ps[:],
                    op0=ALU.mult, op1=ALU.add)
                nc.sync.dma_start(out_d[:], res[:])

    nc.compile()
    return nc


_NC = None


def _get_nc():
    global _NC
    if _NC is None:
        _NC = build_nc()
    return _NC


def make_in_maps(x1, x2):
    x1 = np.asarray(x1, dtype=np.float32)
    x2 = np.asarray(x2, dtype=np.float32)
    x = np.concatenate([x1, x2], axis=0)               # [8192, 256]
    xT8 = np.ascontiguousarray(x.T).astype(ml_dtypes.float8_e4m3fn)
    in_maps = []
    for c in range(8):
        xr = np.roll(xT8, -c * RPC, axis=1)
        in_maps.append(
            {
                "xt0": np.ascontiguousarray(xr[:128]),
                "xt1": np.ascontiguousarray(xr[128:]),
            }
        )
    return in_maps


def _run(x1, x2, trace=False, tmpdir=None):
    from concourse.bass_utils import run_bass_kernel_spmd

    nc = _get_nc()
    in_maps = make_in_maps(x1, x2)
    res = run_bass_kernel_spmd(
        nc, in_maps, list(range(8)), trace=trace, tmpdir=tmpdir
    )
    total = sum(float(res.results[c]["out"][0, 0]) for c in range(8))
    loss = np.asarray(np.float32(total / TWO_N))
    return loss, res


def kernel(x1, x2):
    loss, _ = _run(x1, x2)
    return loss
